# revision 6
# baseline (speedup 1.0000x reference)
"""Trainium2 Bass kernel for nn_LiquidGenerator.

score = sum over (i, image j) pairs of (CUTOFF - dist)^2 where dist < CUTOFF,
with dist over the [N, 27N] supercell distance matrix.

Strategy (v2 — x-sorted per-chunk pruning + fp16 split matmul + fast-mode DVE)
-----------------------------------------------------------------------------
Host (O(N log N) prep):
  * generate P (rotation+translation, float64), sort atoms by x.
  * shift symmetry d(i,(k,j)) == d(j,(-k,i)): central full + 2x the 13
    half-shifts.
  * per-i-chunk j-banding: i-chunks are 128 consecutive x-sorted atoms, so
    each chunk spans a thin x-slab; only columns with x inside
    [slab_min-3, slab_max+3] (and inside the global banding box on y/z for
    shifted images) can contribute. ~960 columns/core total vs 3456 for
    global banding.
  * per-chunk columns are split across the 8 cores round-robin (balanced to
    +-1), then padded per super-chunk to uniform central/shifted widths.
  * distances via a 5-feature inner product, with every fp32 feature split
    into fp16 hi+lo; K=15 rows [hi_l;lo_l;hi_l] x [hi_r;hi_r;lo_r] gives
    ~fp32-accurate d^2 while streaming at 1 PE cycle/column (fp32 matmul
    costs 4).  BIAS keeps the PSUM value strictly positive for ACT Sqrt.

Device (8 NeuronCores): i runs over the 8 chunks as 2 super-chunks of 4;
the four fp16 matmuls of a super-chunk pack into the four 32-row PE groups
(concurrent, one PSUM bank each). Then per super-chunk:
  ScalarE : one Sqrt activation over all 4 banks, PSUM->SBUF fp16
  VectorE : v = min(d,3)-3 via tensor_scalar (4x mode: fp16+SBUF),
            q = v*v via tensor_tensor (2x mode),
            accum via tensor_scalar with accum_out (4x), central (w=1) and
            shifted (w=2, the half-shift weight) as separate accumulators.
(The baseline's scalar_tensor_tensor has no DVE fast mode - 1x - and was the
measured bottleneck; this chain runs the same work in ~1/3 the DVE cycles.)
Host: fp64 sum of partials; exact removal of the device-computed self-pair
terms (recomputed from the read-back fp16 sqrt values) and addition of the
analytic self-pair contribution N*(3-sqrt(EPS))^2.
"""

import numpy as np

CUTOFF = 3.0
EPS = 1e-16
BIAS = 4e-3          # > worst-case d^2 residual of the fp16-split matmul
BAND_MARGIN = 1e-2   # slack on the banding windows

NCORES = 8
N = 1024             # 128 molecules x 8 atoms
NCHUNK = 8           # i-chunks of 128
G = 4                # concurrent PE row groups (chunks per super-chunk)
SC = NCHUNK // G     # super-chunks
TAIL = 128           # zero fp16 tail (bf16-zero operand pool for touchers)
DUMMY_D2 = 100.0

_cache: dict = {}


# ----------------------------------------------------------------- host math
def _rotation_matrices(rot):
    a, b, g = rot[:, 0], rot[:, 1], rot[:, 2]
    ca, sa = np.cos(a), np.sin(a)
    cb, sb = np.cos(b), np.sin(b)
    cg, sg = np.cos(g), np.sin(g)
    m = rot.shape[0]
    rx = np.zeros((m, 3, 3)); ry = np.zeros((m, 3, 3)); rz = np.zeros((m, 3, 3))
    rx[:, 0, 0] = 1;  rx[:, 1, 1] = ca; rx[:, 1, 2] = -sa; rx[:, 2, 1] = sa; rx[:, 2, 2] = ca
    ry[:, 0, 0] = cb; ry[:, 0, 2] = -sb; ry[:, 1, 1] = 1;  ry[:, 2, 0] = sb; ry[:, 2, 2] = cb
    rz[:, 0, 0] = cg; rz[:, 0, 1] = -sg; rz[:, 1, 0] = sg; rz[:, 1, 1] = cg; rz[:, 2, 2] = 1
    return np.einsum("mij,mjk,mkl->mil", rx, ry, rz)


def _generate(positions, translation, rotation, cell):
    R = _rotation_matrices(rotation.astype(np.float64))
    trans = np.remainder(translation.astype(np.float64), 1.0) @ cell.astype(np.float64)
    gen = np.einsum("mai,mij->maj", positions.astype(np.float64), R) + trans[:, None, :]
    return gen.reshape(-1, 3)


def _split16(x):
    """fp32 value -> (hi, lo) fp16 with hi+lo ~ x to ~2^-23 rel."""
    x32 = np.asarray(x, np.float32)
    hi = x32.astype(np.float16)
    lo = (x32 - hi.astype(np.float32)).astype(np.float16)
    return hi, lo


def _lhs_feats(P, c):
    """[5, n] fp32 lhs features [x, y, z, |p|^2, 1] of centered positions."""
    Pc = P - c
    return np.stack([
        Pc[:, 0], Pc[:, 1], Pc[:, 2],
        (Pc ** 2).sum(1),
        np.ones(P.shape[0]),
    ]).astype(np.float32)


def _rhs_feats(S, c, bias):
    """[5, n] fp32 rhs features [-2sx, -2sy, -2sz, 1, |s|^2+bias]."""
    Sc = S - c
    return np.stack([
        -2.0 * Sc[:, 0], -2.0 * Sc[:, 1], -2.0 * Sc[:, 2],
        np.ones(S.shape[0]),
        (Sc ** 2).sum(1) + bias,
    ]).astype(np.float32)


def _pack15(f5):
    """[5, n] fp32 -> ([15, n] fp16 lhs-style, [15, n] fp16 rhs-style)."""
    hi, lo = _split16(f5)
    lhs = np.concatenate([hi, lo, hi], axis=0)
    rhs = np.concatenate([hi, hi, lo], axis=0)
    return lhs, rhs


def _pad8(x):
    return (int(x) + 7) // 8 * 8


# ------------------------------------------------------------- bass program
def _build_program(widths, reps: int = 1, loop_n: int = 0):
    # widths: (W0, C0, W1, C1) - per-super-chunk padded total/central widths.
    key = ("nc", widths, reps, loop_n)
    if key in _cache:
        return _cache[key]
    from contextlib import ExitStack, nullcontext
    import concourse.tile as tile
    from concourse import bacc, mybir

    f32 = mybir.dt.float32
    f16 = mybir.dt.float16
    W0, C0, W1, C1 = widths
    FW = 2 * 128 + W0 + W1 + TAIL

    nc = bacc.Bacc("TRN2", target_bir_lowering=False, debug=False,
                   num_devices=NCORES)
    feat_d = nc.dram_tensor("feat", [128, FW], f16, kind="ExternalInput")
    acc_d = nc.dram_tensor("acc", [128, 4], f32, kind="ExternalOutput")
    sd_d = nc.dram_tensor("sd", [128, 4 * (W0 + W1)], f16,
                          kind="ExternalOutput")

    with tile.TileContext(nc) as tc, ExitStack() as ctx:
        const = ctx.enter_context(tc.tile_pool(name="const", bufs=1))
        psum = ctx.enter_context(tc.tile_pool(name="psum", bufs=2, space="PSUM"))
        vpool = ctx.enter_context(tc.tile_pool(name="v", bufs=2))
        qpool = ctx.enter_context(tc.tile_pool(name="q", bufs=2))
        scrap = ctx.enter_context(tc.tile_pool(name="scrap", bufs=2))

        ft = const.tile([128, FW], f16)
        nc.sync.dma_start(ft[:], feat_d[:])
        at = const.tile([128, 4], f32)
        # persistent sqrt outputs (one per super-chunk) for the self-pair fix
        st0 = const.tile([128, G * W0], f16, tag="st0")
        st1 = const.tile([128, G * W1], f16, tag="st1")
        sts = [st0, st1]

        # bf16-zero views of the zero fp16 tail for "toucher" matmuls
        bw = ft[0:1, FW - TAIL:FW].bitcast(mybir.dt.bfloat16)  # [1,128]
        bx = bw[:, 0:1]

        scinfo = [(W0, C0, 2 * 128), (W1, C1, 2 * 128 + W0)]
        loop_cm = tc.For_i(0, loop_n, 1) if loop_n else nullcontext()
        with loop_cm:
            for u in range(SC * reps):
                s = u % SC
                Ws, Cs, roff = scinfo[s]
                st = sts[s]
                ps = psum.tile([128, G * 512], f32)
                for t in range(G):
                    # row group t handles chunk group[s][t]; the four fp16
                    # matmuls execute concurrently, one PSUM bank each
                    nc.tensor.matmul(
                        ps[:, t * 512:t * 512 + Ws],
                        ft[32 * t:32 * t + 15, s * 128:(s + 1) * 128],
                        ft[32 * t:32 * t + 15, roff:roff + Ws],
                        start=True, stop=True,
                        tile_position=(32 * t, 0),
                    )
                ps3 = ps[:].rearrange("p (g b) -> p g b", g=G)
                st3 = st[:].rearrange("p (g w) -> p g w", g=G)
                # one sqrt over all 4 banks, PSUM -> SBUF fp16
                nc.scalar.activation(st3[:, :, 0:Ws], ps3[:, :, 0:Ws],
                                     mybir.ActivationFunctionType.Sqrt)
                # Toucher: after ACT read the PSUM tile, a 1-column bf16
                # matmul re-takes PSUM ownership on the PE with a single ACT
                # wait, so the next super-chunk's matmuls (which can encode
                # at most one wait) only ever see a same-engine dep.
                nc.tensor.matmul(ps[:, 0:1], bw, bx, start=True, stop=True)
                vt = vpool.tile([128, G * Ws], f16, tag=f"v{s}")
                qt = qpool.tile([128, G * Ws], f16, tag=f"q{s}")
                ot = scrap.tile([128, G * Ws], f16, tag=f"o{s}")
                # v = min(d,3)-3 : tensor_scalar, 4x mode (fp16, all SBUF)
                nc.vector.tensor_scalar(
                    vt[:], st[:], CUTOFF, CUTOFF,
                    mybir.AluOpType.min, mybir.AluOpType.subtract,
                )
                # q = v*v : tensor_tensor, 2x mode
                nc.vector.tensor_tensor(qt[:], vt[:], vt[:],
                                        mybir.AluOpType.mult)
                # accumulate: central (w=1) and shifted (w=2) separately,
                # tensor_scalar with accum_out, 4x mode
                q3 = qt[:].rearrange("p (g w) -> p g w", g=G)
                o3 = ot[:].rearrange("p (g w) -> p g w", g=G)
                nc.vector.tensor_scalar(
                    o3[:, :, 0:Cs], q3[:, :, 0:Cs], 1.0, None,
                    mybir.AluOpType.mult, mybir.AluOpType.add,
                    accum_out=at[:, 2 * s:2 * s + 1],
                )
                nc.vector.tensor_scalar(
                    o3[:, :, Cs:Ws], q3[:, :, Cs:Ws], 2.0, None,
                    mybir.AluOpType.mult, mybir.AluOpType.add,
                    accum_out=at[:, 2 * s + 1:2 * s + 2],
                )
        nc.sync.dma_start(acc_d[:], at[:])
        nc.sync.dma_start(sd_d[:, 0:G * W0], sts[0][:])
        nc.sync.dma_start(sd_d[:, G * W0:G * (W0 + W1)], sts[1][:])

    nc.finalize()
    _cache[key] = nc
    return nc


# --------------------------------------------------------------- input prep
def _prepare_inputs(positions, translation, rotation, cell):
    cell64 = cell.astype(np.float64)
    P = _generate(positions, translation, rotation, cell64)      # [N,3] f64
    n = P.shape[0]
    assert n == N, f"kernel hardcodes N={N}, got {n}"

    shifts = np.array([-1.0, 0.0, 1.0])
    offs = np.stack(np.meshgrid(shifts, shifts, shifts, indexing="ij")).reshape(3, -1).T
    vecs = offs @ cell64                                          # [27,3]
    assert np.all(offs[13] == 0.0)
    half = list(range(13))                                        # pairs with 26-k

    order = np.argsort(P[:, 0], kind="stable")
    P = P[order]
    c = 0.5 * (P.min(axis=0) + P.max(axis=0))

    lo = P.min(axis=0) - (CUTOFF + BAND_MARGIN)
    hi = P.max(axis=0) + (CUTOFF + BAND_MARGIN)

    # per-chunk candidate columns
    JC = N // NCHUNK
    cen_idx, shf_idx = [], []
    for k in range(NCHUNK):
        xlo = P[k * JC:(k + 1) * JC, 0].min() - (CUTOFF + BAND_MARGIN)
        xhi = P[k * JC:(k + 1) * JC, 0].max() + (CUTOFF + BAND_MARGIN)
        cen_idx.append(np.nonzero((P[:, 0] > xlo) & (P[:, 0] < xhi))[0])
        sj, sk = [], []
        for h in half:
            S = P + vecs[h]
            m = (np.all((S > lo) & (S < hi), axis=1)
                 & (S[:, 0] > xlo) & (S[:, 0] < xhi))
            idx = np.nonzero(m)[0]
            sj.append(idx)
            sk.append(np.full(idx.size, h))
        shf_idx.append((np.concatenate(sj), np.concatenate(sk)))

    # group chunks into super-chunks minimizing padded element count
    Cc = [_pad8(np.ceil(ci.size / NCORES)) for ci in cen_idx]
    Sc_ = [_pad8(np.ceil(si[0].size / NCORES)) for si in shf_idx]
    from itertools import combinations
    best, best_g = None, None
    for combo in combinations(range(NCHUNK), G):
        g0 = list(combo); g1 = [k for k in range(NCHUNK) if k not in combo]
        e = (max(Cc[k] for k in g0) + max(Sc_[k] for k in g0)
             + max(Cc[k] for k in g1) + max(Sc_[k] for k in g1))
        if best is None or e < best:
            best, best_g = e, (g0, g1)
    groups = best_g
    C0 = max(Cc[k] for k in groups[0]); S0 = max(Sc_[k] for k in groups[0])
    C1 = max(Cc[k] for k in groups[1]); S1 = max(Sc_[k] for k in groups[1])
    W0, W1 = C0 + S0, C1 + S1
    assert W0 <= 512 and W1 <= 512
    widths = (W0, C0, W1, C1)
    FW = 2 * 128 + W0 + W1 + TAIL

    lhs15, _ = _pack15(_lhs_feats(P, c))                          # [15, N]
    dummy = np.zeros((15, 1), np.float16)
    dummy[4, 0] = DUMMY_D2  # rhs slot: feature 5 pairs with lhs "1"

    # per-core feature images
    feats = []
    for core in range(NCORES):
        feat = np.zeros((128, FW), np.float16)
        for s, grp in enumerate(groups):
            Ws = (W0, W1)[s]
            Cs = (C0, C1)[s]
            roff = 2 * 128 + (0 if s == 0 else W0)
            for t, k in enumerate(grp):
                rows = slice(32 * t, 32 * t + 15)
                # lhs block for this chunk
                feat[rows, s * 128:(s + 1) * 128] = lhs15[:, k * JC:(k + 1) * JC]
                # rhs: centrals then shifted, each padded with dummies
                cen = cen_idx[k][core::NCORES]
                _, rc = _pack15(_rhs_feats(P[cen], c, BIAS))
                sj, sk = shf_idx[k]
                sj, sk = sj[core::NCORES], sk[core::NCORES]
                _, rs = _pack15(_rhs_feats(P[sj] + vecs[sk], c, BIAS))
                blk = np.concatenate([
                    rc, np.repeat(dummy, Cs - rc.shape[1], axis=1),
                    rs, np.repeat(dummy, Ws - Cs - rs.shape[1], axis=1),
                ], axis=1)
                feat[rows, roff:roff + Ws] = blk
        feats.append({"feat": np.ascontiguousarray(feat)})

    # self-pair (diagonal) bookkeeping: atom i of chunk k sits in chunk k's
    # central list; record (core, partition, column in sd) for each
    diag = []  # (core, p, col)
    for s, grp in enumerate(groups):
        Ws = (W0, W1)[s]
        base = 0 if s == 0 else G * W0
        for t, k in enumerate(grp):
            cen = cen_idx[k]
            own = (cen >= k * JC) & (cen < (k + 1) * JC)
            m = np.nonzero(own)[0]
            ii = cen[m]
            assert ii.size == JC  # every chunk atom is inside its own window
            core = m % NCORES
            slot = m // NCORES
            p = ii - k * JC
            col = base + t * Ws + slot
            diag.append((core, p, col))
    diag_core = np.concatenate([d[0] for d in diag])
    diag_p = np.concatenate([d[1] for d in diag])
    diag_col = np.concatenate([d[2] for d in diag])

    meta = {"widths": widths, "diag": (diag_core, diag_p, diag_col)}
    return feats, meta


# ------------------------------------------------------------------- runner
def _get_runner(widths, reps: int = 1, loop_n: int = 0):
    """Jit the bass program once; reuse the compiled executable per call."""
    key = ("runner", widths, reps, loop_n)
    if key in _cache:
        return _cache[key]
    import jax
    from jax.sharding import Mesh, PartitionSpec
    from jax.experimental.shard_map import shard_map
    from concourse import bass2jax, mybir

    nc = _build_program(widths, reps=reps, loop_n=loop_n)
    bass2jax.install_neuronx_cc_hook()

    partition_name = (
        nc.partition_id_tensor.name if nc.partition_id_tensor else None
    )
    in_names, out_names, out_avals, zero_outs = [], [], [], []
    for alloc in nc.m.functions[0].allocations:
        if not isinstance(alloc, mybir.MemoryLocationSet):
            continue
        name = alloc.memorylocations[0].name
        if alloc.kind == "ExternalInput":
            if name != partition_name:
                in_names.append(name)
        elif alloc.kind == "ExternalOutput":
            out_names.append(name)
            shape = tuple(alloc.tensor_shape)
            dtype = mybir.dt.np(alloc.dtype)
            out_avals.append(jax.core.ShapedArray(shape, dtype))
            zero_outs.append(np.zeros(shape, dtype))
    n_params = len(in_names)
    all_in_names = in_names + out_names
    if partition_name is not None:
        all_in_names = all_in_names + [partition_name]

    def _body(*args):
        operands = list(args)
        if partition_name is not None:
            operands.append(bass2jax.partition_id_tensor())
        outs = bass2jax._bass_exec_p.bind(
            *operands,
            out_avals=tuple(out_avals),
            in_names=tuple(all_in_names),
            out_names=tuple(out_names),
            lowering_input_output_aliases=(),
            sim_require_finite=True,
            sim_require_nnan=True,
            nc=nc,
        )
        return tuple(outs)

    devices = jax.devices()[:NCORES]
    mesh = Mesh(np.asarray(devices), ("core",))
    n_outs = len(out_names)
    sharded = jax.jit(
        shard_map(
            _body, mesh=mesh,
            in_specs=(PartitionSpec("core"),) * (n_params + n_outs),
            out_specs=(PartitionSpec("core"),) * n_outs,
            check_rep=False,
        ),
        keep_unused=True,
    )
    concat_zeros = [
        np.zeros((NCORES * z.shape[0], *z.shape[1:]), z.dtype) for z in zero_outs
    ]

    def run(in_maps):
        concat_in = [
            np.concatenate([in_maps[cc][name] for cc in range(NCORES)], axis=0)
            for name in in_names
        ]
        out_arrs = sharded(*concat_in, *concat_zeros)
        return [
            {
                name: np.asarray(out_arrs[i]).reshape(NCORES, *out_avals[i].shape)[cc]
                for i, name in enumerate(out_names)
            }
            for cc in range(NCORES)
        ]

    _cache[key] = run
    return run


def kernel(positions, translation, rotation, cell, _reps=1, _loop_n=0):
    in_maps, meta = _prepare_inputs(
        np.asarray(positions), np.asarray(translation),
        np.asarray(rotation), np.asarray(cell),
    )
    run = _get_runner(meta["widths"], reps=_reps, loop_n=_loop_n)
    results = run(in_maps)
    total = 0.0
    sd = np.stack([r["sd"] for r in results])                # [cores, 128, .]
    for r in results:
        acc = r["acc"].astype(np.float64)
        total += acc.sum()      # central w=1 / shifted w=2 already in accum
    # exact removal of the device-computed self-pair terms, replayed in fp16
    dcore, dp, dcol = meta["diag"]
    d16 = sd[dcore, dp, dcol]                                # fp16 sqrt vals
    v32 = np.minimum(d16.astype(np.float32), np.float32(CUTOFF)) - np.float32(CUTOFF)
    q16 = (v32.astype(np.float16).astype(np.float32) ** 2).astype(np.float16)
    total -= q16.astype(np.float64).sum()
    total += N * (CUTOFF - np.sqrt(np.float32(EPS))) ** 2    # exact self pairs
    return np.float32(total)
